# revision 1
# baseline (speedup 1.0000x reference)
"""Trainium2 Bass kernel for nn_Attention (dense_transformer, ridge regime).

Computation per batch b:
    scores[s]  = <lstm_output[b,s,:], hidden[b,:]>          # [S]
    w          = softmax(scores)                            # [S]
    attn[h]    = sum_s w[s] * lstm_output[b,s,h]            # [H]
    out[b]     = [hidden[b], attn] @ W_combine.T + b_combine

Sharding: data-parallel over batch B=64 across 8 cores (8 batches/core).
W_combine is passed host-transposed (W.T, [2H, H]) and replicated.

Per-core dataflow (all engines explicit, raw bass, one sem wait per instr):
  - DMA: hidden, bias, then L(b) [128,16,1024] per batch (double-buffered),
    then W.T chunks, then the output.
  - PE: replicates hidden[b] across 128 partitions (f32 ones-matmul into
    PSUM), does the weighted sum (einsum2) and final projection as f32r
    matmuls, plus tiny transposes/broadcasts for the softmax reductions.
  - DVE: fused multiply+reduce (tensor_tensor_reduce) computes scores
    directly from the natural [s_partition, h_free] layout -- no transpose
    of the big tensor anywhere.
  - ACT: exp (with -max bias and accumulated partition sums) and the
    normalization-fused copies.
"""

import numpy as np

import concourse.bass as bass
from concourse import bass_isa, library_config, mybir
from concourse.bass_utils import run_bass_kernel_spmd

F32 = mybir.dt.float32
F32R = mybir.dt.float32r

B, S, H = 64, 2048, 1024
NCORES = 8
BPC = B // NCORES          # batches per core
T = S // 128               # s-tiles per batch
NCH = (2 * H) // 128       # 16 chunks of the combined dim
HCH = H // 128             # 8 chunks of one H

_cached_nc = None
last_results = None
PHASE = 6   # debug (55=attnT+ctcp):
SUBV = 0   # phase-5 subvariant 1=scores 2=+maxchain 3=+exp/Z 4=+einsum2/attncopy 5=+attnT/ct 6=full


def _build_program():
    nc = bass.Bass()

    lstm_d = nc.declare_dram_parameter("lstm_output", [BPC, S, H], F32, isOutput=False)
    hid_d = nc.declare_dram_parameter("hidden", [BPC, H], F32, isOutput=False)
    wt_d = nc.declare_dram_parameter("w_t", [2 * H, H], F32, isOutput=False)
    b_d = nc.declare_dram_parameter("b_combine", [H], F32, isOutput=False)
    out_d = nc.declare_dram_parameter("out", [BPC, H], F32, isOutput=True)

    # ---- SBUF ----
    L = [nc.alloc_sbuf_tensor(f"L{i}", [128, T, H], F32R) for i in range(2)]  # 2x8MB
    # W.T reuses L slot 0 at the tail (its DMA starts once einsum2(BPC-2) done)
    WT = L[0]
    hid_t = nc.alloc_sbuf_tensor("hid", [BPC, H], F32)
    hid = hid_t.ap()
    bias_t = nc.alloc_sbuf_tensor("bias", [BPC, H], F32)
    bias = bias_t.ap()
    out_t = nc.alloc_sbuf_tensor("out_sb", [BPC, H], F32)
    out_sb = out_t.ap()
    prod = [nc.alloc_sbuf_tensor(f"prod{i}", [128, H], F32) for i in range(4)]
    dmy = nc.alloc_sbuf_tensor("dmy", [128, T], F32)
    hidR = nc.alloc_sbuf_tensor("hidR", [128, BPC, H], F32)   # 4MB bcast hidden
    CT = nc.alloc_sbuf_tensor("CT", [128, NCH, BPC], F32R)                   # combined^T
    scores = [nc.alloc_sbuf_tensor(f"scores{i}", [128, T], F32) for i in range(2)]
    wexp = [nc.alloc_sbuf_tensor(f"wexp{i}", [128, T], F32R) for i in range(2)]
    zp = [nc.alloc_sbuf_tensor(f"zp{i}", [128, 1], F32) for i in range(2)]
    mp = nc.alloc_sbuf_tensor("mp", [128, 1], F32)
    negM1_t = nc.alloc_sbuf_tensor("negM1s", [1, 2], F32)
    negM1 = [negM1_t.ap()[0:1, i:i + 1] for i in range(2)]
    negM = [nc.alloc_sbuf_tensor(f"negM{i}", [128, 1], F32) for i in range(2)]
    rZ_t = nc.alloc_sbuf_tensor("rZs", [1, 2], F32)
    rZ = [rZ_t.ap()[0:1, i:i + 1] for i in range(2)]
    ones128 = nc.alloc_sbuf_tensor("ones128", [128, 1], F32)
    attn2 = nc.alloc_sbuf_tensor("attn2", [1, 2 * H], F32)
    attn_sb = [attn2.ap()[0:1, i * H:(i + 1) * H] for i in range(2)]
    ones_col = nc.alloc_sbuf_tensor("ones_col", [1, 128], F32)
    ident = nc.alloc_sbuf_tensor("ident", [128, 128], F32)
    sel = nc.alloc_sbuf_tensor("sel", [BPC, BPC, 128], F32)  # sel[k,b,:]=(k==b)

    # ---- PSUM: one bank per concurrent PE write target (the PE wedges on
    # concurrent matmul/transpose-group writes sharing a bank) ----
    acc_lo = nc.alloc_psum_tensor("acc_lo", [BPC, 512], F32)  # einsum2 row 0 / final
    acc_hi = nc.alloc_psum_tensor("acc_hi", [BPC, 512], F32)
    ct8_t = nc.alloc_psum_tensor("ct8", [128, HCH, BPC], F32) # setup transposes
    ctc_t = nc.alloc_psum_tensor("ctc", [128, 512], F32)      # attnT transposes
    stage = nc.alloc_psum_tensor("stage", [128, 512], F32)    # hidR staging mms
    mpT_t = nc.alloc_psum_tensor("mpT", [1, 128], F32)        # transp target
    negM_t = nc.alloc_psum_tensor("negMbc", [128, 1], F32)    # bcast mm target
    Zps_t = nc.alloc_psum_tensor("Zps", [1, 1], F32)          # Z mm target
    mpT = mpT_t.ap()
    negM_bc = negM_t.ap()
    Zps = Zps_t.ap()
    ctcols8 = ct8_t.ap()
    ctcols = ctc_t.ap()[:, 0:HCH]
    stage2 = ctc_t.ap()   # startup-only reuse of the attnT bank

    # ---------------- two-pass emission ----------------
    # ev: event-key -> (sem_name, value).  sems: sem_name -> handle (pass 2).
    ev = {}
    sems = {}
    counts = {}

    class Prog:
        def __init__(self, name):
            self.name = name
            self.emit = False
            self.eng = None
            self.hwm = {}
            # strict-FIFO engines still need pipeline drains between
            # dependent ops for well-defined same-engine ordering
            self.auto_drain = name in ("dve", "act", "gps")
            self.first_op = True

        def begin(self, eng=None, emit=False):
            self.emit = emit
            self.eng = eng
            self.hwm = {}
            self.first_op = True

        def wait(self, key):
            """key: event tuple, or (sem_name, value) pair."""
            if len(key) == 2 and isinstance(key[1], int) and key[0] in (
                    "pe", "dve", "act", "gps", "hid", "bias", "l0", "l1",
                    "wt", "outd", "gdma", "q0", "q1", "q2", "q3"):
                sname, val = key
            else:
                if self.emit and key not in ev:
                    raise KeyError(f"wait on unknown event {key}")
                sname, val = ev.get(key, (None, 0))
            if val <= 0 or sname is None:
                return
            if self.hwm.get(sname, -1) >= val:
                return
            self.hwm[sname] = val
            if self.emit:
                self.eng.wait_ge(sems[sname], val)

        def op(self, fn, inc=1, sem=None, drain=None):
            sname = sem or self.name
            counts[sname] = counts.get(sname, 0) + inc
            if self.emit:
                do_drain = self.auto_drain if drain is None else drain
                if do_drain and not self.first_op:
                    self.eng.drain()
                inst = fn()
                inst.then_inc(sems[sname], inc)
            self.first_op = False

        def mark(self, *key, sem=None):
            sname = sem or self.name
            ev[(self.name,) + tuple(key)] = (sname, counts.get(sname, 0))

    DMA, PE, DVE, ACT, GPS = Prog("dma"), Prog("pe"), Prog("dve"), Prog("act"), Prog("gps")

    bias_src = b_d[:]
    bias_bcast = bass.AP(
        tensor=bias_src.tensor,
        offset=bias_src.offset,
        ap=[[0, BPC]] + list(bias_src.ap),
    )

    def prog_gps():
        g = GPS.eng if GPS.emit else None
        GPS.op(lambda: g.memset(ones_col.ap(), 1.0))
        GPS.op(lambda: g.memset(ones128.ap(), 1.0))
        GPS.op(lambda: g.memset(ident.ap(), 0.0))
        GPS.op(lambda: g.affine_select(
            out=ident.ap(), in_=ident.ap(),
            compare_op=mybir.AluOpType.not_equal, fill=1.0, base=0,
            pattern=[[-1, 128]], channel_multiplier=1))
        GPS.op(lambda: g.memset(sel.ap(), 0.0), drain=True)
        GPS.op(lambda: g.affine_select(
            out=sel.ap(), in_=sel.ap(),
            compare_op=mybir.AluOpType.not_equal, fill=1.0, base=0,
            pattern=[[-1, BPC], [0, 128]], channel_multiplier=1), drain=True)
        GPS.mark("setup")

    def prog_dma():
        d = DMA.eng if DMA.emit else None
        DMA.op(lambda: d.dma_start(out=hid, in_=hid_d[:]), inc=16, sem="hid")
        DMA.mark("hid", sem="hid")
        DMA.op(lambda: d.dma_start(out=bias, in_=bias_bcast), inc=16, sem="bias")
        DMA.mark("bias", sem="bias")
        for b in range(BPC):
            if b >= 2:
                if PHASE >= 4:
                    DMA.wait(("pe", "e2", b - 2))
                else:
                    DMA.wait(("dve", "mult", b - 2, T - 1))
            src = lstm_d[b].rearrange("(t p) h -> p t h", p=128).bitcast(F32R)
            if b == 0:
                # batch 0 arrives in quarters: the first multiplies start
                # ~4x sooner than waiting on the whole 8MB transfer
                for q in range(4):
                    DMA.op(lambda src=src, q=q: d.dma_start(
                        out=L[0].ap()[:, 4 * q:4 * (q + 1), :],
                        in_=src[:, 4 * q:4 * (q + 1), :]),
                        inc=16, sem=f"q{q}")
                    DMA.mark("Lq", q, sem=f"q{q}")
                DMA.mark("L", 0, sem="q3")
            else:
                DMA.op(lambda src=src, b=b: d.dma_start(
                    out=L[b % 2].ap(), in_=src), inc=16, sem=f"l{b % 2}")
                DMA.mark("L", b, sem=f"l{b % 2}")
        if PHASE >= 6:
            DMA.wait(("pe", "e2", BPC - 2))
            wt_src = wt_d[:].rearrange("(c p) n -> p c n", p=128).bitcast(F32R)
            DMA.op(lambda: d.dma_start(out=WT.ap(), in_=wt_src), inc=16, sem="wt")
            DMA.mark("wt", sem="wt")
            DMA.wait(("dve", "bias_hi"))
            out_src = out_sb
        if True:
            pass
        if PHASE < 6:
            gate = {1: ("act", "acc", BPC - 1, T - 1),
                    2: ("dve", "rmax2", BPC - 1),
                    3: ("act", "exp", BPC - 1),
                    4: ("act", "cphi", BPC - 1),
                    5: ("pe", "attnT", BPC - 1),
                    55: ("act", "ctcp", BPC - 1)}[PHASE]
            DMA.wait(gate)
        out_src = out_sb if PHASE >= 6 else hid
        DMA.op(lambda: d.dma_start(out=out_d[:], in_=out_src), inc=16, sem="outd")
        DMA.wait(("outd", counts.get("outd", 0)))

    def prog_pe():
        p = PE.eng if PE.emit else None
        PE.wait(("gps", "setup"))
        PE.wait(("dma", "hid"))
        # hidden^T -> CT chunks 0..7 staging (psum)
        for c in range(HCH):
            PE.op(lambda c=c: p.transpose(
                ctcols8[:, c, :], hid[0:BPC, c * 128:(c + 1) * 128],
                ident.ap()[0:BPC, 0:BPC]))
        PE.mark("hidT")
        # replicate hidden rows across partitions: sel-matmul into the
        # staging bank, DVE/ACT copy out to hidR (all before L(0) lands)
        for k in range(2 * BPC):
            b, j = divmod(k, 2)
            if k == 1:
                PE.wait(("dve", "cth"))   # ctc bank free of setup readers
            if k > 1:
                # wait for the same-bank stage copy two steps back
                pb, pj = divmod(k - 2, 2)
                PE.wait(("dve" if k % 2 == 0 else "act", "hcp", pb, pj))
            tgt = stage.ap() if k % 2 == 0 else stage2
            PE.op(lambda b=b, j=j, tgt=tgt: p.matmul(
                tgt, lhsT=sel.ap()[:, b, :],
                rhs=hid[0:BPC, j * 512:(j + 1) * 512],
                start=True, stop=True))
            PE.mark("hmm", b, j)
        if PHASE >= 2:
            PE.wait(("dve", "rmax", 0))
            PE.op(lambda: p.transpose(mpT, mp.ap(), ident.ap()))
            PE.mark("transp", 0)
        for b in range(BPC):
            if PHASE >= 2:
                PE.wait(("dve", "rmax2", b))
                PE.op(lambda b=b: p.matmul(
                    negM_bc, lhsT=ones_col.ap(), rhs=negM1[b % 2],
                    start=True, stop=True))
                PE.mark("bcast", b)
            if PHASE >= 3:
                if b >= 1:
                    PE.wait(("dve", "recip", b - 1))
                PE.wait(("act", "exp", b))
                PE.op(lambda b=b: p.matmul(
                    Zps, lhsT=zp[b % 2].ap(), rhs=ones128.ap(),
                    start=True, stop=True))
                PE.mark("z", b)
            if PHASE >= 4:
                # einsum2: attn_unnorm = sum_s w[s] * L[s, :]
                PE.wait(("act", "exp", b))
                if b >= 1:
                    PE.wait(("act", "cphi", b - 1))
                for t in range(T):
                    PE.op(lambda b=b, t=t: p.matmul(
                        acc_lo.ap()[0:1, :],
                        lhsT=wexp[b % 2].ap()[:, t:t + 1],
                        rhs=L[b % 2].ap()[:, t, 0:512],
                        start=(t == 0), stop=(t == T - 1)))
                    PE.op(lambda b=b, t=t: p.matmul(
                        acc_hi.ap()[0:1, :],
                        lhsT=wexp[b % 2].ap()[:, t:t + 1],
                        rhs=L[b % 2].ap()[:, t, 512:1024],
                        start=(t == 0), stop=(t == T - 1)))
                PE.mark("e2", b)
            if b + 1 < BPC and PHASE >= 2:
                PE.wait(("dve", "rmax", b + 1))
                PE.op(lambda: p.transpose(mpT, mp.ap(), ident.ap()))
                PE.mark("transp", b + 1)
            if PHASE >= 5:
                # attn row -> columns (chunk transposes via K=1 matmuls)
                PE.wait(("act", "cphi", b))
                if b >= 1 and PHASE in (55, 6):
                    PE.wait(("act", "ctcp", b - 1))
                for c in range(HCH):
                    PE.op(lambda b=b, c=c: p.transpose(
                        ctcols[:, c:c + 1],
                        attn_sb[b % 2][0:1, c * 128:(c + 1) * 128],
                        ones_col.ap()[0:1, 0:1]))
                PE.mark("attnT", b)
        # final projection
        if PHASE < 6:
            return
        PE.wait(("act", "ctcp", BPC - 1))
        PE.wait(("dma", "wt"))
        for c in range(NCH):
            PE.op(lambda c=c: p.matmul(
                acc_lo.ap()[0:BPC, :],
                lhsT=CT.ap()[:, c, :],
                rhs=WT.ap()[:, c, 0:512],
                start=(c == 0), stop=(c == NCH - 1)))
            PE.op(lambda c=c: p.matmul(
                acc_hi.ap()[0:BPC, :],
                lhsT=CT.ap()[:, c, :],
                rhs=WT.ap()[:, c, 512:1024],
                start=(c == 0), stop=(c == NCH - 1)))
        PE.mark("final")

    def prog_dve():
        v = DVE.eng if DVE.emit else None
        # CT hidden columns: psum staging -> CT
        DVE.wait(("pe", "hidT"))
        DVE.op(lambda: v.tensor_copy(CT.ap()[:, 0:HCH, :], ctcols8))
        DVE.mark("cth")
        # startup: copy even hidR stages out of psum (odd ones go to ACT)
        for k in range(0, 2 * BPC, 2):
            b, j = divmod(k, 2)
            DVE.wait(("pe", "hmm", b, j))
            DVE.op(lambda b=b, j=j: v.tensor_copy(
                hidR.ap()[:, b, j * 512:(j + 1) * 512], stage.ap()),
                drain=False)
            DVE.mark("hcp", b, j)
        for b in range(BPC):
            if b > 0:
                DVE.wait(("dma", "L", b))
            DVE.wait(("dve", "hcp", b, 0))
            DVE.wait(("act", "hcp", b, 1))
            for t in range(T):
                # prod slot reuse (8 slots): ACT must have consumed t-8
                NACT = 13
                if b == 0:
                    DVE.wait(("dma", "Lq", t // 4))
                if t >= 4 and t - 4 < NACT:
                    DVE.wait(("act", "acc", b, t - 4))
                elif b >= 1 and t < 4 and T - 4 + t < NACT:
                    DVE.wait(("act", "acc", b - 1, T - 4 + t))
                DVE.op(lambda b=b, t=t: v.tensor_mul(
                    prod[t % 4].ap(),
                    L[b % 2].ap()[:, t, :].bitcast(F32),
                    hidR.ap()[:, b, :]), drain=False)
                DVE.mark("mult", b, t)
                if t >= NACT:
                    if b >= 2:
                        DVE.wait(("act", "exp", b - 2))   # scores slot reuse
                    DVE.op(lambda b=b, t=t: v.reduce_sum(
                        scores[b % 2].ap()[:, t:t + 1], prod[t % 4].ap(),
                        axis=mybir.AxisListType.X))
                    DVE.mark("red", b, t)
                if t == 1 and b >= 1 and PHASE >= 3:
                    # 1/Z of the previous batch, early enough that ACT's
                    # cplo(b-1) (ahead of this batch's accs) can proceed
                    DVE.wait(("pe", "z", b - 1))
                    DVE.op(lambda b=b: v.reciprocal(rZ[(b - 1) % 2], Zps),
                           drain=False)
                    DVE.mark("recip", b - 1)
            DVE.mark("ttr", b)
            if PHASE < 2:
                continue
            if b >= 1:
                DVE.wait(("pe", "transp", b - 1))   # mp slot reuse
            DVE.wait(("act", "acc", b, 12))          # ACT's last score column
            DVE.op(lambda b=b: v.reduce_max(
                mp.ap(), scores[b % 2].ap(), axis=mybir.AxisListType.X))
            DVE.mark("rmax", b)
            DVE.wait(("pe", "transp", b))
            DVE.op(lambda b=b: v.reduce_max(
                negM1[b % 2], mpT, axis=mybir.AxisListType.X, negate=True))
            DVE.mark("rmax2", b)
        if PHASE >= 3:
            DVE.wait(("pe", "z", BPC - 1))
            DVE.op(lambda: v.reciprocal(rZ[(BPC - 1) % 2], Zps))
            DVE.mark("recip", BPC - 1)
        if PHASE < 6:
            return
        # final bias adds
        DVE.wait(("pe", "final"))
        DVE.op(lambda: v.tensor_add(
            out_sb[:, 0:512], acc_lo.ap()[0:BPC, :], bias[:, 0:512]))
        DVE.mark("bias_lo")
        DVE.wait(("dma", "bias"))
        DVE.op(lambda: v.tensor_add(
            out_sb[:, 512:1024], acc_hi.ap()[0:BPC, :], bias[:, 512:1024]))
        DVE.mark("bias_hi")

    def prog_act():
        a = ACT.eng if ACT.emit else None
        Copy = mybir.ActivationFunctionType.Copy
        Exp = mybir.ActivationFunctionType.Exp
        for k in range(1, 2 * BPC, 2):
            b, j = divmod(k, 2)
            ACT.wait(("pe", "hmm", b, j))
            ACT.op(lambda b=b, j=j: a.activation(
                out=hidR.ap()[:, b, j * 512:(j + 1) * 512], in_=stage2,
                func=Copy), drain=False)
            ACT.mark("hcp", b, j)
        for b in range(BPC):
            for t in range(13):
                ACT.wait(("dve", "mult", b, t))
                ACT.op(lambda b=b, t=t: a.activation(
                    out=dmy.ap()[:, t:t + 1].broadcast_to((128, H)),
                    in_=prod[t % 4].ap(),
                    func=Copy, accum_out=scores[b % 2].ap()[:, t:t + 1]),
                    drain=(t == 0))
                ACT.mark("acc", b, t)
            if b >= 2 and PHASE in (55, 6):
                ACT.wait(("pe", "attnT", b - 2))
                ACT.op(lambda b=b: a.activation(
                    out=CT.ap()[:, HCH:NCH, b - 2], in_=ctcols, func=Copy))
                ACT.mark("ctcp", b - 2)
            # previous batch's attn copies -- placed before negMcp/exp so
            # attnT(b-1) -> cphi(b-1) never chains through exp(b)
            if b >= 1 and PHASE >= 4:
                ACT.wait(("pe", "e2", b - 1))
                ACT.wait(("dve", "recip", b - 1))
                ACT.op(lambda b=b: a.activation(
                    out=attn_sb[(b - 1) % 2][0:1, 0:512], in_=acc_lo.ap()[0:1, :],
                    func=Copy, scale=rZ[(b - 1) % 2]))
                ACT.mark("cplo", b - 1)
                ACT.op(lambda b=b: a.activation(
                    out=attn_sb[(b - 1) % 2][0:1, 512:1024], in_=acc_hi.ap()[0:1, :],
                    func=Copy, scale=rZ[(b - 1) % 2]))
                ACT.mark("cphi", b - 1)

            if PHASE >= 2:
                ACT.wait(("pe", "bcast", b))
                ACT.op(lambda b=b: a.activation(
                    out=negM[b % 2].ap(), in_=negM_bc, func=Copy))
                ACT.mark("negMcp", b)
            if PHASE >= 3:
                if b >= 2 and PHASE >= 4:
                    ACT.wait(("pe", "e2", b - 2))    # wexp/zp slot reuse
                ACT.op(lambda b=b: a.activation(
                    out=wexp[b % 2].ap(), in_=scores[b % 2].ap(), func=Exp,
                    bias=negM[b % 2].ap(), scale=1.0, accum_out=zp[b % 2].ap()))
                ACT.mark("exp", b)
            if PHASE < 4:
                continue
        for b in (BPC - 1,):
            ACT.wait(("pe", "e2", b))
            ACT.wait(("dve", "recip", b))
            ACT.op(lambda b=b: a.activation(
                out=attn_sb[b % 2][0:1, 0:512], in_=acc_lo.ap()[0:1, :],
                func=Copy, scale=rZ[b % 2]))
            ACT.mark("cplo", b)
            ACT.op(lambda b=b: a.activation(
                out=attn_sb[b % 2][0:1, 512:1024], in_=acc_hi.ap()[0:1, :],
                func=Copy, scale=rZ[b % 2]))
            ACT.mark("cphi", b)
        if PHASE in (55, 6):
            for b in (BPC - 2, BPC - 1):
                ACT.wait(("pe", "attnT", b))
                ACT.op(lambda b=b: a.activation(
                    out=CT.ap()[:, HCH:NCH, b], in_=ctcols, func=Copy))
                ACT.mark("ctcp", b)

    progs = [
        (GPS, prog_gps), (DMA, prog_dma), (PE, prog_pe),
        (DVE, prog_dve), (ACT, prog_act),
    ]

    # pass 1: count
    for pr, fn in progs:
        pr.begin(emit=False)
        fn()

    # pass 2: emit
    counts.clear()
    sem_names = ["pe", "dve", "act", "gps", "hid", "bias", "l0", "l1", "wt",
                 "outd", "gdma", "q0", "q1", "q2", "q3"]
    with nc.Block() as block:
        for sn in sem_names:
            sems[sn] = nc.alloc_semaphore(name=f"{sn}_sem")

        @block.gpsimd
        def _(eng):
            GPS.begin(eng=eng, emit=True)
            prog_gps()

        @block.sync
        def _(eng):
            DMA.begin(eng=eng, emit=True)
            prog_dma()

        @block.tensor
        def _(eng):
            PE.begin(eng=eng, emit=True)
            prog_pe()

        @block.vector
        def _(eng):
            DVE.begin(eng=eng, emit=True)
            prog_dve()

        @block.scalar
        def _(eng):
            ACT.begin(eng=eng, emit=True)
            prog_act()

    return nc


def kernel(lstm_output, hidden, W_combine, b_combine):
    global _cached_nc, last_results
    lstm_output = np.asarray(lstm_output, dtype=np.float32)
    hidden = np.asarray(hidden, dtype=np.float32)
    W_combine = np.asarray(W_combine, dtype=np.float32)
    b_combine = np.asarray(b_combine, dtype=np.float32)

    if _cached_nc is None:
        _cached_nc = _build_program()
    nc = _cached_nc

    wt_host = np.ascontiguousarray(W_combine.T)
    in_maps = []
    for i in range(NCORES):
        sl = slice(i * BPC, (i + 1) * BPC)
        in_maps.append({
            "lstm_output": np.ascontiguousarray(lstm_output[sl]),
            "hidden": np.ascontiguousarray(hidden[sl]),
            "w_t": wt_host,
            "b_combine": b_combine,
        })
    res = run_bass_kernel_spmd(nc, in_maps, core_ids=list(range(NCORES)))
    last_results = res
    return np.concatenate([res.results[i]["out"] for i in range(NCORES)], axis=0)



# revision 5
# speedup vs baseline: 1.4110x; 1.4110x over previous
"""Trainium2 Bass kernel for nn_Attention (dense_transformer, ridge regime).

Computation per batch b:
    scores[s]  = <lstm_output[b,s,:], hidden[b,:]>          # [S]
    w          = softmax(scores)                            # [S]
    attn[h]    = sum_s w[s] * lstm_output[b,s,h]            # [H]
    out[b]     = [hidden[b], attn] @ W_combine.T + b_combine

Sharding: data-parallel over batch B=64 across 8 cores (8 batches/core).
W_combine is passed host-transposed (W.T, [2H, H]) and replicated.

v3 (bf16, all-engine balance): all large operands are cast to bf16 on the
host, halving HBM traffic.  Measured per-[128,1024]-tile costs drove the
split: DVE fused mult 0.55us, any reduce ~1.2us, ACT accum 1.41us, GPS mult
2.56us, PE matmul[*,512] 0.38+0.09us.

Per-core dataflow, per batch (16 s-tiles):
  - DMA: L(b) [128,16,1024] bf16, p-major (contiguous per partition),
    double-buffered, issued in 1MiB quarters; W.T after L(1).
  - products L*hidR: DVE fused-8 mult (tiles 0-7), fused-6 (8-13); GPS
    tensor_mul (14-15).
  - row-sums -> scores: DVE reduce_sum (tiles 0-5), ACT Copy+accum (6-15).
  - softmax: DVE rmax -> PE transpose -> DVE rmax2(neg) -> PE bcast ->
    ACT copy -> ACT exp (bias=-max, Z accumulated per partition).
  - einsum2 on PE: M=8 matmuls, lhsT = per-batch zero-padded [128,8] column
    block of wexp, accumulating all 8 batches into one persistent PSUM pair;
    per-batch Z matmul.
  - projection: hidden half spread over mid-stream PE slack, attn half at
    the tail after the end-of-stream attn transposes.
"""

import numpy as np
import ml_dtypes

import concourse.bass as bass
from concourse import bass_isa, library_config, mybir
from concourse.bass_utils import run_bass_kernel_spmd

F32 = mybir.dt.float32
BF16 = mybir.dt.bfloat16
NPBF16 = ml_dtypes.bfloat16

B, S, H = 64, 2048, 1024
NCORES = 8
BPC = B // NCORES          # batches per core
T = S // 128               # s-tiles per batch
NCH = (2 * H) // 128       # 16 chunks of the combined dim
HCH = H // 128             # 8 chunks of one H

NDVE_MUL = 14              # tiles 0..13 multiplied on DVE (rest on GPS)
NDVE_RED = 6               # tiles 0..5 reduced on DVE (rest on ACT)

_cached_nc = None
last_results = None


def _build_program():
    nc = bass.Bass()

    lstm_d = nc.declare_dram_parameter("lstm_output", [BPC, S, H], BF16, isOutput=False)
    hid_d = nc.declare_dram_parameter("hidden", [BPC, H], BF16, isOutput=False)
    wt_d = nc.declare_dram_parameter("w_t", [2 * H, H], BF16, isOutput=False)
    b_d = nc.declare_dram_parameter("b_combine", [H], F32, isOutput=False)
    out_d = nc.declare_dram_parameter("out", [BPC, H], F32, isOutput=True)

    # ---- SBUF ----
    L = [nc.alloc_sbuf_tensor(f"L{i}", [128, T, H], BF16) for i in range(2)]
    WT = nc.alloc_sbuf_tensor("WT", [128, NCH, H], BF16)
    hid_t = nc.alloc_sbuf_tensor("hid", [BPC, H], BF16)
    hid = hid_t.ap()
    bias_t = nc.alloc_sbuf_tensor("bias", [BPC, H], F32)
    bias = bias_t.ap()
    out_t = nc.alloc_sbuf_tensor("out_sb", [BPC, H], F32)
    out_sb = out_t.ap()
    hidR = nc.alloc_sbuf_tensor("hidR", [128, BPC, H], BF16)
    prodP = [nc.alloc_sbuf_tensor(f"prodP{i}", [128, NDVE_MUL, H], BF16)
             for i in range(2)]
    prodG = [nc.alloc_sbuf_tensor(f"prodG{i}", [128, T - NDVE_MUL, H], BF16)
             for i in range(2)]
    dmy = nc.alloc_sbuf_tensor("dmy", [128, 1], BF16)
    CT = nc.alloc_sbuf_tensor("CT", [128, NCH, BPC], BF16)
    wexpP = [nc.alloc_sbuf_tensor(f"wexpP{b}", [128, T, BPC], BF16)
             for b in range(BPC)]
    scores = [nc.alloc_sbuf_tensor(f"scores{b}", [128, T], F32) for b in range(BPC)]
    mp = [nc.alloc_sbuf_tensor(f"mp{b}", [128, 1], F32) for b in range(BPC)]
    zp = [nc.alloc_sbuf_tensor(f"zp{b}", [128, 1], F32) for b in range(BPC)]
    negM = [nc.alloc_sbuf_tensor(f"negM{b}", [128, 1], F32) for b in range(BPC)]
    negM1_t = nc.alloc_sbuf_tensor("negM1s", [1, BPC], F32)
    negM1 = [negM1_t.ap()[0:1, b:b + 1] for b in range(BPC)]
    attn8 = nc.alloc_sbuf_tensor("attn8", [BPC, H], BF16)
    rZrow_t = nc.alloc_sbuf_tensor("rZrow", [1, BPC], F32)
    rZrow = rZrow_t.ap()
    rZv_t = nc.alloc_sbuf_tensor("rZv", [BPC, 1], F32)
    rZv = rZv_t.ap()
    ones_col = nc.alloc_sbuf_tensor("ones_col", [1, 128], F32)
    ones128 = nc.alloc_sbuf_tensor("ones128", [128, 1], F32)
    ident = nc.alloc_sbuf_tensor("ident", [128, 128], F32)
    identB = nc.alloc_sbuf_tensor("identB", [128, 128], BF16)
    sel = nc.alloc_sbuf_tensor("sel", [BPC, BPC, 128], BF16)

    # ---- PSUM: 8 banks ----
    e2lo_t = nc.alloc_psum_tensor("e2lo", [BPC, 512], F32)
    e2hi_t = nc.alloc_psum_tensor("e2hi", [BPC, 512], F32)
    pjlo_t = nc.alloc_psum_tensor("pjlo", [BPC, 512], F32)
    pjhi_t = nc.alloc_psum_tensor("pjhi", [BPC, 512], F32)
    stage_t = nc.alloc_psum_tensor("stage", [128, 512], F32)
    stage2_t = nc.alloc_psum_tensor("stage2", [128, 512], F32)
    mpT_t = nc.alloc_psum_tensor("mpT", [1, 128], F32)
    zbank_t = nc.alloc_psum_tensor("zbank", [BPC, 64], F32)
    e2lo, e2hi = e2lo_t.ap(), e2hi_t.ap()
    pjlo, pjhi = pjlo_t.ap(), pjhi_t.ap()
    stage, stage2 = stage_t.ap(), stage2_t.ap()
    mpT = mpT_t.ap()
    negM_bc = stage2_t.ap()[:, 0:1]
    Zps = zbank_t.ap()[0:1, 0:BPC]
    rZvT = zbank_t.ap()[0:BPC, 8:9]
    ctT = [stage_t.ap()[:, 4 * c:4 * (c + 1)].bitcast(BF16) for c in range(HCH)]

    # ---------------- two-pass emission ----------------
    ev = {}
    sems = {}
    counts = {}

    class Prog:
        def __init__(self, name):
            self.name = name
            self.emit = False
            self.eng = None
            self.hwm = {}
            self.auto_drain = name in ("dve", "act", "gps")
            self.first_op = True

        def begin(self, eng=None, emit=False):
            self.emit = emit
            self.eng = eng
            self.hwm = {}
            self.first_op = True

        def wait(self, key):
            if len(key) == 2 and isinstance(key[1], int) and key[0] in (
                    "pe", "dve", "act", "gps", "hid", "bias",
                    "l0", "l1", "wt", "outd"):
                sname, val = key
            else:
                if self.emit and key not in ev:
                    raise KeyError(f"wait on unknown event {key}")
                sname, val = ev.get(key, (None, 0))
            if val <= 0 or sname is None:
                return
            if self.hwm.get(sname, -1) >= val:
                return
            self.hwm[sname] = val
            if self.emit:
                self.eng.wait_ge(sems[sname], val)

        def op(self, fn, inc=1, sem=None, drain=None):
            sname = sem or self.name
            counts[sname] = counts.get(sname, 0) + inc
            if self.emit:
                do_drain = self.auto_drain if drain is None else drain
                if do_drain and not self.first_op:
                    self.eng.drain()
                inst = fn()
                inst.then_inc(sems[sname], inc)
            self.first_op = False

        def mark(self, *key, sem=None):
            sname = sem or self.name
            ev[(self.name,) + tuple(key)] = (sname, counts.get(sname, 0))

    DMA, PE, DVE, ACT, GPS = Prog("dma"), Prog("pe"), Prog("dve"), Prog("act"), Prog("gps")

    bias_src = b_d[:]
    bias_bcast = bass.AP(
        tensor=bias_src.tensor,
        offset=bias_src.offset,
        ap=[[0, BPC]] + list(bias_src.ap),
    )

    def prog_gps():
        g = GPS.eng if GPS.emit else None
        GPS.op(lambda: g.memset(ones_col.ap(), 1.0))
        GPS.op(lambda: g.memset(ones128.ap(), 1.0))
        GPS.op(lambda: g.memset(ident.ap(), 0.0))
        GPS.op(lambda: g.affine_select(
            out=ident.ap(), in_=ident.ap(),
            compare_op=mybir.AluOpType.not_equal, fill=1.0, base=0,
            pattern=[[-1, 128]], channel_multiplier=1))
        GPS.op(lambda: g.memset(identB.ap(), 0.0), drain=True)
        GPS.op(lambda: g.affine_select(
            out=identB.ap(), in_=identB.ap(),
            compare_op=mybir.AluOpType.not_equal, fill=1.0, base=0,
            pattern=[[-1, 128]], channel_multiplier=1), drain=True)
        GPS.op(lambda: g.memset(sel.ap(), 0.0), drain=True)
        GPS.op(lambda: g.affine_select(
            out=sel.ap(), in_=sel.ap(),
            compare_op=mybir.AluOpType.not_equal, fill=1.0, base=0,
            pattern=[[-1, BPC], [0, 128]], channel_multiplier=1), drain=True)
        for b in range(BPC):
            GPS.op(lambda b=b: g.memset(wexpP[b].ap(), 0.0), drain=False)
        GPS.mark("setup")
        # per-batch: products for tiles NDVE_MUL..15
        for b in range(BPC):
            GPS.wait(("dma", "Lq", b, 3))
            GPS.wait(("dve", "hcp", b, 0))
            GPS.wait(("act", "hcp", b, 1))
            if b >= 2:
                GPS.wait(("act", "red", b - 2))   # prodG slot reuse
            for j in range(T - NDVE_MUL):
                GPS.op(lambda b=b, j=j: g.tensor_mul(
                    prodG[b % 2].ap()[:, j, :],
                    L[b % 2].ap()[:, NDVE_MUL + j, :],
                    hidR.ap()[:, b, :]), drain=False)
            GPS.mark("gmul", b)

    def prog_dma():
        d = DMA.eng if DMA.emit else None
        DMA.op(lambda: d.dma_start(out=hid, in_=hid_d[:]), inc=16, sem="hid")
        DMA.mark("hid", sem="hid")
        DMA.op(lambda: d.dma_start(out=bias, in_=bias_bcast), inc=16, sem="bias")
        DMA.mark("bias", sem="bias")
        for b in range(BPC):
            if b >= 2:
                DMA.wait(("pe", "e2", b - 2))
            src = lstm_d[b].rearrange("(p t) h -> p t h", t=T)
            sl = f"l{b % 2}"
            for q in range(4):
                DMA.op(lambda src=src, b=b, q=q: d.dma_start(
                    out=L[b % 2].ap()[:, 4 * q:4 * (q + 1), :],
                    in_=src[:, 4 * q:4 * (q + 1), :]),
                    inc=16, sem=sl)
                DMA.mark("Lq", b, q, sem=sl)
            DMA.mark("L", b, sem=sl)
            if b == 1:
                wt_src = wt_d[:].rearrange("(c p) n -> p c n", p=128)
                DMA.op(lambda: d.dma_start(out=WT.ap(), in_=wt_src),
                       inc=16, sem="wt")
                DMA.mark("wt", sem="wt")
        DMA.wait(("dve", "bias_hi"))
        DMA.op(lambda: d.dma_start(out=out_d[:], in_=out_sb), inc=16, sem="outd")
        DMA.wait(("outd", counts.get("outd", 0)))

    def prog_pe():
        p = PE.eng if PE.emit else None
        PE.wait(("gps", "setup"))
        PE.wait(("dma", "hid"))
        for c in range(HCH):
            PE.op(lambda c=c: p.transpose(
                ctT[c], hid[0:BPC, c * 128:(c + 1) * 128],
                identB.ap()[0:BPC, 0:BPC]))
        PE.mark("hidT")
        for k in range(2 * BPC):
            b, j = divmod(k, 2)
            if k == 0:
                PE.wait(("dve", "cth"))
            if k > 1:
                pb, pj = divmod(k - 2, 2)
                PE.wait(("dve" if k % 2 == 0 else "act", "hcp", pb, pj))
            tgt = stage if k % 2 == 0 else stage2
            PE.op(lambda b=b, j=j, tgt=tgt: p.matmul(
                tgt, lhsT=sel.ap()[:, b, :],
                rhs=hid[0:BPC, j * 512:(j + 1) * 512],
                start=True, stop=True))
            PE.mark("hmm", b, j)
        for b in range(BPC):
            PE.wait(("dve", "rmax", b))
            if b >= 1:
                PE.wait(("dve", "rmax2", b - 1))
            PE.op(lambda b=b: p.transpose(mpT, mp[b].ap(), ident.ap()))
            PE.mark("transp", b)
            PE.wait(("dve", "rmax2", b))
            if b == 0:
                PE.wait(("act", "hcp", BPC - 1, 1))
            else:
                PE.wait(("act", "negMcp", b - 1))
            PE.op(lambda b=b: p.matmul(
                negM_bc, lhsT=ones_col.ap(), rhs=negM1[b],
                start=True, stop=True))
            PE.mark("bcast", b)
            PE.wait(("act", "exp", b))
            PE.op(lambda b=b: p.matmul(
                Zps[0:1, b:b + 1], lhsT=zp[b].ap(), rhs=ones128.ap(),
                start=True, stop=True, skip_group_check=True))
            PE.mark("z", b)
            PE.wait(("dma", "L", b))
            for t in range(T):
                PE.op(lambda b=b, t=t: p.matmul(
                    e2lo[0:BPC, :],
                    lhsT=wexpP[b].ap()[:, t, :],
                    rhs=L[b % 2].ap()[:, t, 0:512],
                    start=(b == 0 and t == 0), stop=(b == BPC - 1 and t == T - 1),
                    skip_group_check=True))
                PE.op(lambda b=b, t=t: p.matmul(
                    e2hi[0:BPC, :],
                    lhsT=wexpP[b].ap()[:, t, :],
                    rhs=L[b % 2].ap()[:, t, 512:1024],
                    start=(b == 0 and t == 0), stop=(b == BPC - 1 and t == T - 1),
                    skip_group_check=True))
            PE.mark("e2", b)
            if 2 <= b <= 5:
                PE.wait(("dma", "wt"))
                PE.wait(("dve", "cth"))
                for c in (2 * (b - 2), 2 * (b - 2) + 1):
                    PE.op(lambda c=c: p.matmul(
                        pjlo[0:BPC, :], lhsT=CT.ap()[:, c, :],
                        rhs=WT.ap()[:, c, 0:512],
                        start=(c == 0), stop=False, skip_group_check=True))
                    PE.op(lambda c=c: p.matmul(
                        pjhi[0:BPC, :], lhsT=CT.ap()[:, c, :],
                        rhs=WT.ap()[:, c, 512:1024],
                        start=(c == 0), stop=False, skip_group_check=True))
                PE.mark("pjh", b)
        # ---- tail ----
        PE.wait(("dve", "recip"))
        PE.op(lambda: p.transpose(rZvT, rZrow, ones128.ap()[0:1, 0:1]))
        PE.mark("rZvT")
        PE.wait(("act", "cphi"))
        PE.wait(("dve", "hcp", BPC - 1, 0))
        for c in range(HCH):
            PE.op(lambda c=c: p.transpose(
                ctT[c], attn8.ap()[0:BPC, c * 128:(c + 1) * 128],
                identB.ap()[0:BPC, 0:BPC]))
        PE.mark("attnT")
        PE.wait(("dve", "ctA"))
        for c in range(HCH, NCH):
            PE.op(lambda c=c: p.matmul(
                pjlo[0:BPC, :], lhsT=CT.ap()[:, c, :],
                rhs=WT.ap()[:, c, 0:512],
                start=False, stop=(c == NCH - 1), skip_group_check=True))
            PE.op(lambda c=c: p.matmul(
                pjhi[0:BPC, :], lhsT=CT.ap()[:, c, :],
                rhs=WT.ap()[:, c, 512:1024],
                start=False, stop=(c == NCH - 1), skip_group_check=True))
        PE.mark("projdone")

    def prog_dve():
        v = DVE.eng if DVE.emit else None
        DVE.wait(("pe", "hidT"))
        DVE.op(lambda: v.tensor_copy(
            CT.ap()[:, 0:HCH, :], stage_t.ap()[:, 0:4 * HCH].bitcast(BF16)))
        DVE.mark("cth")
        for k in range(0, 2 * BPC, 2):
            b, j = divmod(k, 2)
            DVE.wait(("pe", "hmm", b, j))
            DVE.op(lambda b=b, j=j: v.tensor_copy(
                hidR.ap()[:, b, j * 512:(j + 1) * 512], stage), drain=False)
            DVE.mark("hcp", b, j)
        for b in range(BPC):
            DVE.wait(("act", "hcp", b, 1))
            if b >= 2:
                DVE.wait(("act", "red", b - 2))   # prodP slot reuse
            DVE.wait(("dma", "Lq", b, 1))
            h8 = hidR.ap()[:, b, :].unsqueeze(1).broadcast_to((128, 8, H))
            DVE.op(lambda b=b, h8=h8: v.tensor_mul(
                prodP[b % 2].ap()[:, 0:8, :], L[b % 2].ap()[:, 0:8, :], h8),
                drain=False)
            DVE.mark("multA", b)
            DVE.wait(("dma", "Lq", b, 3))
            h6 = hidR.ap()[:, b, :].unsqueeze(1).broadcast_to(
                (128, NDVE_MUL - 8, H))
            DVE.op(lambda b=b, h6=h6: v.tensor_mul(
                prodP[b % 2].ap()[:, 8:NDVE_MUL, :],
                L[b % 2].ap()[:, 8:NDVE_MUL, :], h6), drain=False)
            DVE.mark("multB", b)
            for t in range(NDVE_RED):
                DVE.op(lambda b=b, t=t: v.reduce_sum(
                    scores[b].ap()[:, t:t + 1], prodP[b % 2].ap()[:, t, :],
                    axis=mybir.AxisListType.X), drain=False)
            DVE.mark("redD", b)
            DVE.wait(("act", "red", b))
            DVE.op(lambda b=b: v.reduce_max(
                mp[b].ap(), scores[b].ap(), axis=mybir.AxisListType.X))
            DVE.mark("rmax", b)
            DVE.wait(("pe", "transp", b))
            DVE.op(lambda b=b: v.reduce_max(
                negM1[b], mpT, axis=mybir.AxisListType.X, negate=True))
            DVE.mark("rmax2", b)
        # ---- tail ----
        DVE.wait(("pe", "z", BPC - 1))
        DVE.op(lambda: v.reciprocal(rZrow, Zps))
        DVE.mark("recip")
        DVE.wait(("pe", "attnT"))
        DVE.op(lambda: v.tensor_copy(
            CT.ap()[:, HCH:NCH, :], stage_t.ap()[:, 0:4 * HCH].bitcast(BF16)))
        DVE.mark("ctA")
        DVE.wait(("pe", "projdone"))
        DVE.wait(("dma", "bias"))
        DVE.op(lambda: v.tensor_add(out_sb[:, 0:512], pjlo[0:BPC, :], bias[:, 0:512]))
        DVE.mark("bias_lo")
        DVE.op(lambda: v.tensor_add(out_sb[:, 512:1024], pjhi[0:BPC, :],
                                    bias[:, 512:1024]), drain=False)
        DVE.mark("bias_hi")

    def prog_act():
        a = ACT.eng if ACT.emit else None
        Copy = mybir.ActivationFunctionType.Copy
        Exp = mybir.ActivationFunctionType.Exp
        for k in range(1, 2 * BPC, 2):
            b, j = divmod(k, 2)
            ACT.wait(("pe", "hmm", b, j))
            ACT.op(lambda b=b, j=j: a.activation(
                out=hidR.ap()[:, b, j * 512:(j + 1) * 512], in_=stage2,
                func=Copy), drain=False)
            ACT.mark("hcp", b, j)
        for b in range(BPC):
            # reductions for tiles NDVE_RED..15
            ACT.wait(("dve", "multB", b))
            for t in range(NDVE_RED, NDVE_MUL):
                ACT.op(lambda b=b, t=t: a.activation(
                    out=dmy.ap().broadcast_to((128, H)),
                    in_=prodP[b % 2].ap()[:, t, :], func=Copy,
                    accum_out=scores[b].ap()[:, t:t + 1]),
                    drain=(t == NDVE_RED))
            ACT.wait(("gps", "gmul", b))
            for j in range(T - NDVE_MUL):
                ACT.op(lambda b=b, j=j: a.activation(
                    out=dmy.ap().broadcast_to((128, H)),
                    in_=prodG[b % 2].ap()[:, j, :], func=Copy,
                    accum_out=scores[b].ap()[:, NDVE_MUL + j:NDVE_MUL + j + 1]),
                    drain=False)
            ACT.mark("red", b)
            ACT.wait(("pe", "bcast", b))
            ACT.op(lambda b=b: a.activation(
                out=negM[b].ap(), in_=negM_bc, func=Copy))
            ACT.mark("negMcp", b)
            ACT.op(lambda b=b: a.activation(
                out=wexpP[b].ap()[:, :, b], in_=scores[b].ap(), func=Exp,
                bias=negM[b].ap(), scale=1.0, accum_out=zp[b].ap()))
            ACT.mark("exp", b)
        # ---- tail ----
        ACT.wait(("pe", "rZvT"))
        ACT.op(lambda: a.activation(out=rZv, in_=rZvT, func=Copy))
        ACT.mark("rzv")
        ACT.wait(("pe", "e2", BPC - 1))
        ACT.op(lambda: a.activation(
            out=attn8.ap()[0:BPC, 0:512], in_=e2lo[0:BPC, :],
            func=Copy, scale=rZv))
        ACT.mark("cplo")
        ACT.op(lambda: a.activation(
            out=attn8.ap()[0:BPC, 512:1024], in_=e2hi[0:BPC, :],
            func=Copy, scale=rZv), drain=False)
        ACT.mark("cphi")

    progs = [
        (GPS, prog_gps), (DMA, prog_dma), (PE, prog_pe),
        (DVE, prog_dve), (ACT, prog_act),
    ]

    for pr, fn in progs:
        pr.begin(emit=False)
        fn()

    counts.clear()
    sem_names = ["pe", "dve", "act", "gps", "hid", "bias",
                 "l0", "l1", "wt", "outd"]
    with nc.Block() as block:
        for sn in sem_names:
            sems[sn] = nc.alloc_semaphore(name=f"{sn}_sem")

        @block.gpsimd
        def _(eng):
            GPS.begin(eng=eng, emit=True)
            prog_gps()

        @block.sync
        def _(eng):
            DMA.begin(eng=eng, emit=True)
            prog_dma()

        @block.tensor
        def _(eng):
            PE.begin(eng=eng, emit=True)
            prog_pe()

        @block.vector
        def _(eng):
            DVE.begin(eng=eng, emit=True)
            prog_dve()

        @block.scalar
        def _(eng):
            ACT.begin(eng=eng, emit=True)
            prog_act()

    return nc


def kernel(lstm_output, hidden, W_combine, b_combine):
    global _cached_nc, last_results
    lstm_output = np.asarray(lstm_output, dtype=np.float32)
    hidden = np.asarray(hidden, dtype=np.float32)
    W_combine = np.asarray(W_combine, dtype=np.float32)
    b_combine = np.asarray(b_combine, dtype=np.float32)

    if _cached_nc is None:
        _cached_nc = _build_program()
    nc = _cached_nc

    wt_host = np.ascontiguousarray(W_combine.T).astype(NPBF16)
    in_maps = []
    for i in range(NCORES):
        sl = slice(i * BPC, (i + 1) * BPC)
        in_maps.append({
            "lstm_output": np.ascontiguousarray(lstm_output[sl]).astype(NPBF16),
            "hidden": np.ascontiguousarray(hidden[sl]).astype(NPBF16),
            "w_t": wt_host,
            "b_combine": b_combine,
        })
    res = run_bass_kernel_spmd(nc, in_maps, core_ids=list(range(NCORES)))
    last_results = res
    return np.concatenate([res.results[i]["out"] for i in range(NCORES)], axis=0)
